# revision 14
# baseline (speedup 1.0000x reference)
"""KAN layer (B=8192, IN_F=OUT_F=1024, GRID=5) on 8 Trainium2 cores.

Math: Y[b,o] = W0[o]*silu(x) + spline_o(clip(x,-1,1)) * W1[o] + b[o], x = X[b,o]
(idx_in = arange(O) % IN_F is the identity here since O == IN_F).

Factorization used here (clip-form basis, exact):
  Y = W0*silu(x) + B'*xc + G1*M1 + G2*M2 + G3*M3 + A''
  xc  = clip(x, -1, 1)
  Mj  = clip(x, s_j, 1),  s_j in {-0.5, 0.0, 0.5}
  B'  = w1*sl0;  Gj = w1*(sl_j - sl_{j-1});  sl_g = 2*(c_{g+1}-c_g)
  A'' = w1*(c0 + sl0 + 0.5*d1 - 0.5*d3) + b   (d_j = sl_j - sl_{j-1})

Sharding: EDGES across the 8 cores (128 edges per core, full batch 8192 on
the free dim).  X arrives pre-transposed and cast to fp16 on host
([128 edges, 8192 batch]); output returns as fp16 and is cast back on host
(tolerance is 2e-2; fp16 I/O costs ~1e-3).  The 5 diagonal stationaries are
pre-built on the host ([128, 5, 128] fp16) so no DVE work gates the first
matmul.

Per 512-col chunk (one PSUM bank) the per-edge weighted sum runs on
TensorE as diagonal fp16 matmuls accumulating in PSUM.  AFF_GROUPS
subgroups skip the M3 matmul: their evacuation runs on DVE as
affine_then_add (yo = M3*G3 + A'' + psum); other chunks evacuate on
ScalarE (Identity + per-edge A'' bias) after a 5th matmul.  silu on
ScalarE, xc/M1/M3 on DVE (tensor_scalar, 4x fp16 mode), M2 on GpSimd
for g>0 and on DVE for group 0.

Queue plan (the DMA wire is a shared ~240GB/s pipe but the two HWDGE
queues order independently): all x loads + wp on the Sync queue (x0
split in two 512-col pieces so the first chunk computes earlier); wpd
(diag stack) on the ScalarE queue.  y stores are per 512-col chunk,
routed so their semaphore wait never blocks compute issue: chunks
evacuated on ScalarE store from the ScalarE queue (the wait is on the
instruction just before them), chunks evacuated on DVE store from the
Sync queue (which has no compute to block).  Evacuations are emitted
BEFORE the next subgroup's features so the DVE/ACT queues run them as
soon as the PSUM bank closes -- stores then enter the wire by data
readiness instead of piling up behind the load backlog.  Warmup matmuls
on scratch open the PE clock gate until the first real matmul; the last
subgroup splits its evacuation across DVE+ACT for a short tail.
"""
import sys

for _p in ("/root/.axon_site", "/root/.axon_site/_ro/trn_rl_repo", "/root/.axon_site/_ro/pypackages"):
    if _p not in sys.path:
        sys.path.append(_p)

import numpy as np

import concourse.bacc as bacc
import concourse.tile as tile
from concourse import mybir
from concourse.bass_utils import run_bass_kernel_spmd

B, IN_F, OUT_F, GRID = 8192, 1024, 1024, 5
N_CORES = 8
E_SHARD = OUT_F // N_CORES      # 128 edges per core
NG = 8                          # batch groups of 1024
GW = B // NG                    # group width (1024)
CHUNK = 512                     # one PSUM bank of fp32
N_WARM = 8                      # PE clock-gate warmup matmuls
AFF_GROUPS = (0, 2, 4, 6)       # subgroups evacuated via DVE affine_then_add
X_BUFS = 8                      # all x tiles resident

_nc_cache = None


def _build():
    f16 = mybir.dt.float16
    f32 = mybir.dt.float32
    AF = mybir.ActivationFunctionType
    OP = mybir.AluOpType
    nc = bacc.Bacc("TRN2", target_bir_lowering=False, debug=False)
    xt = nc.dram_tensor("xt", [E_SHARD, B], f16, kind="ExternalInput").ap()
    wp = nc.dram_tensor("wp", [E_SHARD, 8], f32, kind="ExternalInput").ap()
    wpd = nc.dram_tensor("wpd", [E_SHARD, 5 * E_SHARD], f16,
                         kind="ExternalInput").ap()
    yt = nc.dram_tensor("yt", [E_SHARD, B], f16, kind="ExternalOutput").ap()

    with tile.TileContext(nc) as tc:
        with tc.tile_pool(name="const", bufs=1) as cpool, \
             tc.tile_pool(name="xin", bufs=X_BUFS) as xpool, \
             tc.tile_pool(name="x0h", bufs=2) as x0pool, \
             tc.tile_pool(name="fsil", bufs=3) as spool, \
             tc.tile_pool(name="fxc", bufs=3) as xcpool, \
             tc.tile_pool(name="fm1", bufs=3) as m1pool, \
             tc.tile_pool(name="fm2", bufs=3) as m2pool, \
             tc.tile_pool(name="fm3", bufs=3) as m3pool, \
             tc.tile_pool(name="yout", bufs=3) as ypool, \
             tc.tile_pool(name="ps", bufs=7, space="PSUM") as pspool, \
             tc.tile_pool(name="pswarm", bufs=1, space="PSUM") as wpool:

            scr = cpool.tile([128, CHUNK], f16)
            dum = cpool.tile([128, 16], f16)

            # --- input prefetch on the SP queue: x0 gates the whole
            # pipeline so it goes first, split in two 512-col halves.
            # wp (per-edge scalars) rides between x0 and x1; it is only
            # needed by the first affine evac (~2us after the first mm).
            x0h = []
            for hh in range(2):
                t_ = x0pool.tile([128, CHUNK], f16, tag=f"x0{hh}", name=f"x0{hh}")
                nc.sync.dma_start(t_[:], xt[:, hh * CHUNK:(hh + 1) * CHUNK])
                x0h.append(t_)
            wpt = cpool.tile([128, 8], f32)
            nc.sync.dma_start(wpt[:], wp[:, :])
            g3c = wpt[:, 4:5]
            apc = wpt[:, 5:6]
            xg = [None]
            for g in range(1, NG):
                x_t = xpool.tile([128, GW], f16, tag=f"x{g}", name=f"x{g}")
                nc.sync.dma_start(x_t[:], xt[:, g * GW:(g + 1) * GW])
                xg.append(x_t)

            # --- diag stack on the ScalarE queue (its only DMA); the ACT
            # table loads for Silu+Identity are hoisted by the compiler
            # and hide behind the x0 DMA wait.
            wpdt = cpool.tile([128, 5, E_SHARD], f16)
            nc.scalar.dma_start(wpdt[:], wpd[:, :])
            diag = wpdt

            # --- engine warmups on private scratch + PE clock-gate opener.
            # The warm matmuls only exist to burn PE cycles (their PSUM
            # bank is reset by start=True before its one real use for the
            # last chunk).
            nc.vector.memset(scr[:], 0.0)
            nc.vector.tensor_scalar(dum[:, 2:3], dum[:, 2:3],
                                    1.0, -1.0, OP.min, OP.max)
            nc.gpsimd.tensor_scalar(dum[:, 5:6], dum[:, 5:6],
                                    1.0, -1.0, OP.min, OP.max)
            ps_warm = wpool.tile([128, CHUNK], f32, tag="pswarm", name="pswarm")
            for _ in range(N_WARM):
                nc.tensor.matmul(ps_warm[:], scr[:, 0:128], scr[:],
                                 start=True, stop=True, skip_group_check=True)

            feats = {}      # subgroup -> feature tiles (g0: per-half lists)

            def emit_features0():
                sil, xc, m1, m2, m3 = [], [], [], [], []
                for hh in range(2):
                    xv = x0h[hh][:]
                    s_ = spool.tile([128, CHUNK], f16, tag="sil", name=f"sil0{hh}")
                    nc.scalar.activation(s_[:], xv, AF.Silu)
                    sil.append(s_)
                    c_ = xcpool.tile([128, CHUNK], f16, tag="xc", name=f"xc0{hh}")
                    nc.vector.tensor_scalar(c_[:], xv, 1.0, -1.0, OP.min, OP.max)
                    xc.append(c_)
                    a_ = m1pool.tile([128, CHUNK], f16, tag="m1", name=f"m10{hh}")
                    nc.vector.tensor_scalar(a_[:], xv, 1.0, -0.5, OP.min, OP.max)
                    m1.append(a_)
                    b_ = m2pool.tile([128, CHUNK], f16, tag="m2", name=f"m20{hh}")
                    nc.vector.tensor_scalar(b_[:], xv, 1.0, 0.0, OP.min, OP.max)
                    m2.append(b_)
                    d_ = m3pool.tile([128, CHUNK], f16, tag="m3", name=f"m30{hh}")
                    nc.vector.tensor_scalar(d_[:], xv, 1.0, 0.5, OP.min, OP.max)
                    m3.append(d_)
                feats[0] = (sil, xc, m1, m2, m3)

            def emit_features(g):
                xv = xg[g][:]
                sil = spool.tile([128, GW], f16, tag="sil", name=f"sil{g}")
                nc.scalar.activation(sil[:], xv, AF.Silu)
                xc = xcpool.tile([128, GW], f16, tag="xc", name=f"xc{g}")
                nc.vector.tensor_scalar(xc[:], xv, 1.0, -1.0, OP.min, OP.max)
                m1 = m1pool.tile([128, GW], f16, tag="m1", name=f"m1{g}")
                nc.vector.tensor_scalar(m1[:], xv, 1.0, -0.5, OP.min, OP.max)
                m2 = m2pool.tile([128, GW], f16, tag="m2", name=f"m2{g}")
                nc.gpsimd.tensor_scalar(m2[:], xv, 1.0, 0.0, OP.min, OP.max)
                m3 = m3pool.tile([128, GW], f16, tag="m3", name=f"m3{g}")
                nc.vector.tensor_scalar(m3[:], xv, 1.0, 0.5, OP.min, OP.max)
                feats[g] = (sil, xc, m1, m2, m3)

            def emit_subgroup(g):
                """matmuls + evac + per-chunk store for 1024-col subgroup g."""
                sil, xc, m1, m2, m3 = feats.pop(g)
                yo = ypool.tile([128, GW], f16, tag="yo", name=f"yo{g}")
                last = (g == NG - 1)
                aff_g = g in AFF_GROUPS
                for h in range(2):
                    if g == 0:
                        fsil, fxc, fm1 = sil[h][:], xc[h][:], m1[h][:]
                        fm2, fm3 = m2[h][:], m3[h][:]
                    else:
                        cs = slice(h * CHUNK, (h + 1) * CHUNK)
                        fsil, fxc, fm1 = sil[:, cs], xc[:, cs], m1[:, cs]
                        fm2, fm3 = m2[:, cs], m3[:, cs]
                    if last and h == 1:
                        # the very last chunk takes the long-retired warm bank
                        pc = wpool.tile([128, CHUNK], f32, tag="pswarm",
                                        name="ps_last")[:]
                    else:
                        pc = pspool.tile([128, CHUNK], f32, tag="ps",
                                         name=f"ps{g}_{h}")[:]
                    # last subgroup: h0 evacs on DVE, h1 on ACT (concurrent
                    # short tail; identity is the faster final evac)
                    aff = (aff_g and not last) or (last and h == 0)
                    nc.tensor.matmul(pc, diag[:, 1, :], fxc,
                                     start=True, stop=False, skip_group_check=True)
                    nc.tensor.matmul(pc, diag[:, 2, :], fm1,
                                     start=False, stop=False, skip_group_check=True)
                    nc.tensor.matmul(pc, diag[:, 3, :], fm2,
                                     start=False, stop=False, skip_group_check=True)
                    if not aff:
                        nc.tensor.matmul(pc, diag[:, 4, :], fm3,
                                         start=False, stop=False,
                                         skip_group_check=True)
                    nc.tensor.matmul(pc, diag[:, 0, :], fsil,
                                     start=False, stop=True, skip_group_check=True)
                    ye = slice(h * CHUNK, (h + 1) * CHUNK)
                    yd = yt[:, g * GW + h * CHUNK:g * GW + (h + 1) * CHUNK]
                    if aff:
                        nc.vector.affine_then_add(yo[:, ye], fm3, pc,
                                                  scale=g3c, bias=apc)
                        # DVE-evacuated chunk: store from the Sync queue (no
                        # compute there for the cross-engine wait to block)
                        nc.sync.dma_start(yd, yo[:, ye])
                    else:
                        nc.scalar.activation(yo[:, ye], pc, AF.Identity,
                                             bias=apc, scale=1.0)
                        # ACT-evacuated chunk: store from the ScalarE queue;
                        # its wait is on the Identity just before it
                        nc.scalar.dma_start(yd, yo[:, ye])

            # software-pipelined emission: features are emitted AFTER the
            # subgroup two behind them, so evac ops sit ahead of later
            # feature ops in the DVE/ACT queues and PSUM banks close early;
            # the feature DATA still leads the matmul stream by 2 subgroups.
            emit_features0()
            emit_features(1)
            for g in range(NG):
                emit_subgroup(g)
                if g + 2 < NG:
                    emit_features(g + 2)
    nc.compile()
    return nc


def _host_prep(X, coeffs, W, b):
    c = coeffs.astype(np.float64)
    Wd = W.astype(np.float64)
    bd = b.astype(np.float64)
    sl = 2.0 * (c[:, 1:] - c[:, :-1])           # [O, 4] segment slopes
    d = sl[:, 1:] - sl[:, :-1]                  # [O, 3] slope deltas at knots
    w1 = Wd[:, 1]
    bprime = w1 * sl[:, 0]
    g = w1[:, None] * d                         # [O, 3]
    app = w1 * (c[:, 0] + sl[:, 0] + 0.5 * d[:, 0] - 0.5 * d[:, 2]) + bd

    wp = np.zeros((OUT_F, 8), dtype=np.float32)
    wp[:, 0] = Wd[:, 0]
    wp[:, 1] = bprime
    wp[:, 2] = g[:, 0]
    wp[:, 3] = g[:, 1]
    wp[:, 4] = g[:, 2]
    wp[:, 5] = app
    return wp


def _in_maps(X, coeffs, W, b):
    wp = _host_prep(X, coeffs, W, b)
    in_maps = []
    for cid in range(N_CORES):
        sl = slice(cid * E_SHARD, (cid + 1) * E_SHARD)
        xt = np.ascontiguousarray(X[:, sl].T.astype(np.float16))
        wpc = np.ascontiguousarray(wp[sl])
        # pre-built diagonal stationaries: [E, 5, E], order
        # 0=W0(silu) 1=B'(xc) 2=G1(M1) 3=G2(M2) 4=G3(M3)
        wpd = np.zeros((E_SHARD, 5, E_SHARD), dtype=np.float16)
        idx = np.arange(E_SHARD)
        for f in range(5):
            wpd[idx, f, idx] = wpc[:, f].astype(np.float16)
        in_maps.append({"xt": xt, "wp": wpc,
                        "wpd": np.ascontiguousarray(wpd.reshape(E_SHARD, -1))})
    return in_maps


def kernel(X, coeffs, W, b):
    global _nc_cache
    if _nc_cache is None:
        _nc_cache = _build()
    nc = _nc_cache

    in_maps = _in_maps(X, coeffs, W, b)
    res = run_bass_kernel_spmd(nc, in_maps, core_ids=list(range(N_CORES)))
    Y = np.empty((B, OUT_F), dtype=np.float32)
    for c in range(N_CORES):
        sl = slice(c * E_SHARD, (c + 1) * E_SHARD)
        Y[:, sl] = res.results[c]["yt"].T.astype(np.float32)
    return Y


# revision 17
# speedup vs baseline: 1.0178x; 1.0178x over previous
"""KAN layer (B=8192, IN_F=OUT_F=1024, GRID=5) on 8 Trainium2 cores.

Math: Y[b,o] = W0[o]*silu(x) + spline_o(clip(x,-1,1)) * W1[o] + b[o], x = X[b,o]
(idx_in = arange(O) % IN_F is the identity here since O == IN_F).

Factorization used here (clip-form basis, exact):
  Y = W0*silu(x) + B'*xc + G1*M1 + G2*M2 + G3*M3 + A''
  xc  = clip(x, -1, 1)
  Mj  = clip(x, s_j, 1),  s_j in {-0.5, 0.0, 0.5}
  B'  = w1*sl0;  Gj = w1*(sl_j - sl_{j-1});  sl_g = 2*(c_{g+1}-c_g)
  A'' = w1*(c0 + sl0 + 0.5*d1 - 0.5*d3) + b   (d_j = sl_j - sl_{j-1})

Sharding: EDGES across the 8 cores (128 edges per core, full batch 8192 on
the free dim).  X arrives pre-transposed and cast to fp16 on host
([128 edges, 8192 batch]); output returns as fp16 and is cast back on host
(tolerance is 2e-2; fp16 I/O costs ~1e-3).  The 5 diagonal stationaries are
pre-built on the host ([128, 5, 128] fp16) so no DVE work gates the first
matmul.

Per 512-col chunk (one PSUM bank) the per-edge weighted sum runs on
TensorE as diagonal fp16 matmuls accumulating in PSUM.  AFF_GROUPS
subgroups skip the M3 matmul: their evacuation runs on DVE as
affine_then_add (yo = M3*G3 + A'' + psum); other chunks evacuate on
ScalarE (Identity + per-edge A'' bias) after a 5th matmul.  silu on
ScalarE, xc/M1/M3 on DVE (tensor_scalar, 4x fp16 mode), M2 on GpSimd
for g>0 and on DVE for group 0.

Queue plan (the DMA wire is a shared ~240GB/s pipe but the two HWDGE
queues order independently): all x loads + wp on the Sync queue (x0
split in two 512-col pieces so the first chunk computes earlier); wpd
(diag stack) on the ScalarE queue.  y stores are per 512-col chunk,
routed so their semaphore wait never blocks compute issue: chunks
evacuated on ScalarE store from the ScalarE queue (the wait is on the
instruction just before them), chunks evacuated on DVE store from the
Sync queue (which has no compute to block).  Evacuations are emitted
BEFORE the next subgroup's features so the DVE/ACT queues run them as
soon as the PSUM bank closes -- stores then enter the wire by data
readiness instead of piling up behind the load backlog.  Warmup matmuls
on scratch open the PE clock gate until the first real matmul; the last
subgroup splits its evacuation across DVE+ACT for a short tail.
"""
import sys

for _p in ("/root/.axon_site", "/root/.axon_site/_ro/trn_rl_repo", "/root/.axon_site/_ro/pypackages"):
    if _p not in sys.path:
        sys.path.append(_p)

import numpy as np

import concourse.bacc as bacc
import concourse.tile as tile
from concourse import mybir
from concourse.bass_utils import run_bass_kernel_spmd

B, IN_F, OUT_F, GRID = 8192, 1024, 1024, 5
N_CORES = 8
E_SHARD = OUT_F // N_CORES      # 128 edges per core
NG = 8                          # batch groups of 1024
GW = B // NG                    # group width (1024)
CHUNK = 512                     # one PSUM bank of fp32
N_WARM = 6                      # PE clock-gate warmup matmuls
AFF_GROUPS = (0, 2, 4, 6)       # subgroups evacuated via DVE affine_then_add
X_BUFS = 8                      # all x tiles resident

_nc_cache = None


def _build():
    f16 = mybir.dt.float16
    f32 = mybir.dt.float32
    AF = mybir.ActivationFunctionType
    OP = mybir.AluOpType
    nc = bacc.Bacc("TRN2", target_bir_lowering=False, debug=False)
    xt = nc.dram_tensor("xt", [E_SHARD, B], f16, kind="ExternalInput").ap()
    wp = nc.dram_tensor("wp", [E_SHARD, 8], f32, kind="ExternalInput").ap()
    wpd = nc.dram_tensor("wpd", [E_SHARD, 5 * E_SHARD], f16,
                         kind="ExternalInput").ap()
    yt = nc.dram_tensor("yt", [E_SHARD, B], f16, kind="ExternalOutput").ap()

    with tile.TileContext(nc) as tc:
        with tc.tile_pool(name="const", bufs=1) as cpool, \
             tc.tile_pool(name="xin", bufs=X_BUFS) as xpool, \
             tc.tile_pool(name="x0h", bufs=2) as x0pool, \
             tc.tile_pool(name="fsil", bufs=3) as spool, \
             tc.tile_pool(name="fxc", bufs=3) as xcpool, \
             tc.tile_pool(name="fm1", bufs=3) as m1pool, \
             tc.tile_pool(name="fm2", bufs=3) as m2pool, \
             tc.tile_pool(name="fm3", bufs=3) as m3pool, \
             tc.tile_pool(name="yout", bufs=3) as ypool, \
             tc.tile_pool(name="ps", bufs=7, space="PSUM") as pspool, \
             tc.tile_pool(name="pswarm", bufs=1, space="PSUM") as wpool:

            scr = cpool.tile([128, CHUNK], f16)
            dum = cpool.tile([128, 16], f16)

            # --- input prefetch on the SP queue: x0 gates the whole
            # pipeline so it goes first, split in two 512-col halves.
            # wp (per-edge scalars) rides between x0 and x1; it is only
            # needed by the first affine evac (~2us after the first mm).
            x0h = []
            for hh in range(2):
                t_ = x0pool.tile([128, CHUNK], f16, tag=f"x0{hh}", name=f"x0{hh}")
                nc.sync.dma_start(t_[:], xt[:, hh * CHUNK:(hh + 1) * CHUNK])
                x0h.append(t_)
            wpt = cpool.tile([128, 8], f32)
            nc.sync.dma_start(wpt[:], wp[:, :])
            g3c = wpt[:, 4:5]
            apc = wpt[:, 5:6]
            xg = [None]
            for g in range(1, NG):
                x_t = xpool.tile([128, GW], f16, tag=f"x{g}", name=f"x{g}")
                nc.sync.dma_start(x_t[:], xt[:, g * GW:(g + 1) * GW])
                xg.append(x_t)

            # --- diag stack on the ScalarE queue (its only DMA); the ACT
            # table loads for Silu+Identity are hoisted by the compiler
            # and hide behind the x0 DMA wait.
            wpdt = cpool.tile([128, 5, E_SHARD], f16)
            nc.scalar.dma_start(wpdt[:], wpd[:, :])
            diag = wpdt

            # --- engine warmups on private scratch + PE clock-gate opener.
            # The warm matmuls only exist to burn PE cycles (their PSUM
            # bank is reset by start=True before its one real use for the
            # last chunk).
            nc.vector.memset(scr[:], 0.0)
            nc.vector.tensor_scalar(dum[:, 2:3], dum[:, 2:3],
                                    1.0, -1.0, OP.min, OP.max)
            nc.gpsimd.tensor_scalar(dum[:, 5:6], dum[:, 5:6],
                                    1.0, -1.0, OP.min, OP.max)
            ps_warm = wpool.tile([128, CHUNK], f32, tag="pswarm", name="pswarm")
            for _ in range(N_WARM):
                nc.tensor.matmul(ps_warm[:], scr[:, 0:128], scr[:],
                                 start=True, stop=True, skip_group_check=True)

            feats = {}      # subgroup -> feature tiles (g0: per-half lists)

            def emit_features0():
                sil, xc, m1, m2, m3 = [], [], [], [], []
                for hh in range(2):
                    xv = x0h[hh][:]
                    s_ = spool.tile([128, CHUNK], f16, tag="sil", name=f"sil0{hh}")
                    nc.scalar.activation(s_[:], xv, AF.Silu)
                    sil.append(s_)
                    c_ = xcpool.tile([128, CHUNK], f16, tag="xc", name=f"xc0{hh}")
                    nc.vector.tensor_scalar(c_[:], xv, 1.0, -1.0, OP.min, OP.max)
                    xc.append(c_)
                    a_ = m1pool.tile([128, CHUNK], f16, tag="m1", name=f"m10{hh}")
                    nc.vector.tensor_scalar(a_[:], xv, 1.0, -0.5, OP.min, OP.max)
                    m1.append(a_)
                    b_ = m2pool.tile([128, CHUNK], f16, tag="m2", name=f"m20{hh}")
                    nc.vector.tensor_scalar(b_[:], xv, 1.0, 0.0, OP.min, OP.max)
                    m2.append(b_)
                    d_ = m3pool.tile([128, CHUNK], f16, tag="m3", name=f"m30{hh}")
                    nc.vector.tensor_scalar(d_[:], xv, 1.0, 0.5, OP.min, OP.max)
                    m3.append(d_)
                feats[0] = (sil, xc, m1, m2, m3)

            def emit_features(g):
                xv = xg[g][:]
                sil = spool.tile([128, GW], f16, tag="sil", name=f"sil{g}")
                nc.scalar.activation(sil[:], xv, AF.Silu)
                xc = xcpool.tile([128, GW], f16, tag="xc", name=f"xc{g}")
                nc.vector.tensor_scalar(xc[:], xv, 1.0, -1.0, OP.min, OP.max)
                m1 = m1pool.tile([128, GW], f16, tag="m1", name=f"m1{g}")
                nc.vector.tensor_scalar(m1[:], xv, 1.0, -0.5, OP.min, OP.max)
                m2 = m2pool.tile([128, GW], f16, tag="m2", name=f"m2{g}")
                nc.gpsimd.tensor_scalar(m2[:], xv, 1.0, 0.0, OP.min, OP.max)
                m3 = m3pool.tile([128, GW], f16, tag="m3", name=f"m3{g}")
                nc.vector.tensor_scalar(m3[:], xv, 1.0, 0.5, OP.min, OP.max)
                feats[g] = (sil, xc, m1, m2, m3)

            def emit_subgroup(g):
                """matmuls + evac + per-chunk store for 1024-col subgroup g."""
                sil, xc, m1, m2, m3 = feats.pop(g)
                yo = ypool.tile([128, GW], f16, tag="yo", name=f"yo{g}")
                last = (g == NG - 1)
                aff_g = g in AFF_GROUPS
                for h in range(2):
                    if g == 0:
                        fsil, fxc, fm1 = sil[h][:], xc[h][:], m1[h][:]
                        fm2, fm3 = m2[h][:], m3[h][:]
                    else:
                        cs = slice(h * CHUNK, (h + 1) * CHUNK)
                        fsil, fxc, fm1 = sil[:, cs], xc[:, cs], m1[:, cs]
                        fm2, fm3 = m2[:, cs], m3[:, cs]
                    if last and h == 1:
                        # the very last chunk takes the long-retired warm bank
                        pc = wpool.tile([128, CHUNK], f32, tag="pswarm",
                                        name="ps_last")[:]
                    else:
                        pc = pspool.tile([128, CHUNK], f32, tag="ps",
                                         name=f"ps{g}_{h}")[:]
                    # last subgroup: h0 evacs on DVE, h1 on ACT (concurrent
                    # short tail; identity is the faster final evac)
                    aff = (aff_g and not last) or (last and h == 0)
                    # matmul order: for g>=1 consume the ACT/GpSimd-produced
                    # features (immune to the DMA-wire SBUF contention that
                    # drops DVE clips to 1x) first, extending the DVE clip
                    # deadline by two matmul slots
                    if g == 0:
                        order = [(1, fxc), (2, fm1), (3, fm2), (0, fsil)]
                    else:
                        order = [(0, fsil), (3, fm2), (1, fxc), (2, fm1)]
                    if not aff:
                        order.append((4, fm3))
                    for k, (fi, ft) in enumerate(order):
                        nc.tensor.matmul(pc, diag[:, fi, :], ft,
                                         start=(k == 0), stop=(k == len(order) - 1),
                                         skip_group_check=True)
                    ye = slice(h * CHUNK, (h + 1) * CHUNK)
                    if aff:
                        nc.vector.affine_then_add(yo[:, ye], fm3, pc,
                                                  scale=g3c, bias=apc)
                    else:
                        nc.scalar.activation(yo[:, ye], pc, AF.Identity,
                                             bias=apc, scale=1.0)
                    if last:
                        # split last-group stores for a short tail
                        nc.sync.dma_start(yt[:, g * GW + h * CHUNK:
                                             g * GW + (h + 1) * CHUNK],
                                          yo[:, ye])
                if not last:
                    nc.sync.dma_start(yt[:, g * GW:(g + 1) * GW], yo[:])

            # software-pipelined emission: features run two subgroups ahead
            # of the matmul/evac stream so a slow feature op never stalls PE
            emit_features0()
            emit_features(1)
            for g in range(2, NG):
                emit_features(g)
                emit_subgroup(g - 2)
            emit_subgroup(NG - 2)
            emit_subgroup(NG - 1)
    nc.compile()
    return nc


def _host_prep(X, coeffs, W, b):
    c = coeffs.astype(np.float64)
    Wd = W.astype(np.float64)
    bd = b.astype(np.float64)
    sl = 2.0 * (c[:, 1:] - c[:, :-1])           # [O, 4] segment slopes
    d = sl[:, 1:] - sl[:, :-1]                  # [O, 3] slope deltas at knots
    w1 = Wd[:, 1]
    bprime = w1 * sl[:, 0]
    g = w1[:, None] * d                         # [O, 3]
    app = w1 * (c[:, 0] + sl[:, 0] + 0.5 * d[:, 0] - 0.5 * d[:, 2]) + bd

    wp = np.zeros((OUT_F, 8), dtype=np.float32)
    wp[:, 0] = Wd[:, 0]
    wp[:, 1] = bprime
    wp[:, 2] = g[:, 0]
    wp[:, 3] = g[:, 1]
    wp[:, 4] = g[:, 2]
    wp[:, 5] = app
    return wp


def _in_maps(X, coeffs, W, b):
    wp = _host_prep(X, coeffs, W, b)
    in_maps = []
    for cid in range(N_CORES):
        sl = slice(cid * E_SHARD, (cid + 1) * E_SHARD)
        xt = np.ascontiguousarray(X[:, sl].T.astype(np.float16))
        wpc = np.ascontiguousarray(wp[sl])
        # pre-built diagonal stationaries: [E, 5, E], order
        # 0=W0(silu) 1=B'(xc) 2=G1(M1) 3=G2(M2) 4=G3(M3)
        wpd = np.zeros((E_SHARD, 5, E_SHARD), dtype=np.float16)
        idx = np.arange(E_SHARD)
        for f in range(5):
            wpd[idx, f, idx] = wpc[:, f].astype(np.float16)
        in_maps.append({"xt": xt, "wp": wpc,
                        "wpd": np.ascontiguousarray(wpd.reshape(E_SHARD, -1))})
    return in_maps


def kernel(X, coeffs, W, b):
    global _nc_cache
    if _nc_cache is None:
        _nc_cache = _build()
    nc = _nc_cache

    in_maps = _in_maps(X, coeffs, W, b)
    res = run_bass_kernel_spmd(nc, in_maps, core_ids=list(range(N_CORES)))
    Y = np.empty((B, OUT_F), dtype=np.float32)
    for c in range(N_CORES):
        sl = slice(c * E_SHARD, (c + 1) * E_SHARD)
        Y[:, sl] = res.results[c]["yt"].T.astype(np.float32)
    return Y


# revision 21
# speedup vs baseline: 1.0415x; 1.0233x over previous
"""KAN layer (B=8192, IN_F=OUT_F=1024, GRID=5) on 8 Trainium2 cores.

Math: Y[b,o] = W0[o]*silu(x) + spline_o(clip(x,-1,1)) * W1[o] + b[o], x = X[b,o]
(idx_in = arange(O) % IN_F is the identity here since O == IN_F).

Factorization used here (clip-form basis, exact):
  Y = W0*silu(x) + B'*xc + G1*M1 + G2*M2 + G3*M3 + A''
  xc  = clip(x, -1, 1)
  Mj  = clip(x, s_j, 1),  s_j in {-0.5, 0.0, 0.5}
  B'  = w1*sl0;  Gj = w1*(sl_j - sl_{j-1});  sl_g = 2*(c_{g+1}-c_g)
  A'' = w1*(c0 + sl0 + 0.5*d1 - 0.5*d3) + b   (d_j = sl_j - sl_{j-1})

Sharding: EDGES across the 8 cores (128 edges per core, full batch 8192 on
the free dim).  X arrives pre-transposed and cast to fp16 on host
([128 edges, 8192 batch]); output returns as fp16 and is cast back on host
(tolerance is 2e-2; fp16 I/O costs ~1e-3).  The 5 diagonal stationaries
are pre-built on the host ([128, 5, 128] fp16) so no on-device work gates
the first matmul.

Per 512-col chunk (one PSUM bank) the per-edge weighted sum runs on
TensorE as diagonal fp16 matmuls accumulating in PSUM.  AFF_GROUPS
subgroups skip the M3 matmul: their evacuation runs on DVE as
affine_then_add (yo = M3*G3 + A'' + psum); other chunks evacuate on
ScalarE (Identity + per-edge A'' bias) after a 5th matmul.  silu on
ScalarE, xc/M1/M3 on DVE (tensor_scalar, 4x fp16 mode), M2 on GpSimd for
g>1 and on DVE for groups 0-1.

Queue plan: all x loads + wp ride the Sync HWDGE queue with the first two
batch groups split in 512-col halves (x0a x0b x1a x1b wp x2..x7) so the
pipeline head starts ~1us earlier; wpd (diag stack) rides the ScalarE
queue ahead of the ACT table loads.  y stores ride Sync as subgroup pairs
(the wire is a shared ~240GB/s pipe; they drain behind the loads).
Warmup ops on private scratch open the PE clock gate and preload both
ACT table sets during the x0 DMA wait; the last subgroup splits its
evacuation across DVE+ACT and its final stores in 256-col pieces for a
short tail.
"""
import sys

for _p in ("/root/.axon_site", "/root/.axon_site/_ro/trn_rl_repo", "/root/.axon_site/_ro/pypackages"):
    if _p not in sys.path:
        sys.path.append(_p)

import numpy as np

import concourse.bacc as bacc
import concourse.tile as tile
from concourse import mybir
from concourse.bass_utils import run_bass_kernel_spmd

B, IN_F, OUT_F, GRID = 8192, 1024, 1024, 5
N_CORES = 8
E_SHARD = OUT_F // N_CORES      # 128 edges per core
NG = 8                          # batch groups of 1024
GW = B // NG                    # group width (1024)
CHUNK = 512                     # one PSUM bank of fp32
N_WARM = 7                      # PE clock-gate warmup matmuls
AFF_GROUPS = (0, 2, 4, 6)       # subgroups evacuated via DVE affine_then_add
SPLIT_GROUPS = (0, 1)           # subgroups whose x loads/features are 512-col

_nc_cache = None


def _build():
    f16 = mybir.dt.float16
    f32 = mybir.dt.float32
    AF = mybir.ActivationFunctionType
    OP = mybir.AluOpType
    nc = bacc.Bacc("TRN2", target_bir_lowering=False, debug=False)
    xt = nc.dram_tensor("xt", [E_SHARD, B], f16, kind="ExternalInput").ap()
    wp = nc.dram_tensor("wp", [E_SHARD, 8], f32, kind="ExternalInput").ap()
    wpd = nc.dram_tensor("wpd", [E_SHARD, 5 * E_SHARD], f16,
                         kind="ExternalInput").ap()
    yt = nc.dram_tensor("yt", [E_SHARD, B], f16, kind="ExternalOutput").ap()

    with tile.TileContext(nc) as tc:
        with tc.tile_pool(name="const", bufs=1) as cpool, \
             tc.tile_pool(name="xin", bufs=NG - len(SPLIT_GROUPS)) as xpool, \
             tc.tile_pool(name="xsp", bufs=2 * len(SPLIT_GROUPS)) as xsplit, \
             tc.tile_pool(name="fsil", bufs=3) as spool, \
             tc.tile_pool(name="fxc", bufs=3) as xcpool, \
             tc.tile_pool(name="fm1", bufs=3) as m1pool, \
             tc.tile_pool(name="fm2", bufs=3) as m2pool, \
             tc.tile_pool(name="fm3", bufs=3) as m3pool, \
             tc.tile_pool(name="fspl", bufs=10 * len(SPLIT_GROUPS)) as fsplit, \
             tc.tile_pool(name="yout", bufs=3) as ypool, \
             tc.tile_pool(name="ps", bufs=7, space="PSUM") as pspool, \
             tc.tile_pool(name="pswarm", bufs=1, space="PSUM") as wpool:

            scr = cpool.tile([128, CHUNK], f16)
            dum = cpool.tile([128, 16], f16)

            # --- input prefetch on the SP queue.  First two groups split
            # in 512-col halves so the pipeline head starts sooner; wp
            # (per-edge scalars, needed first by the g0 affine evac) rides
            # between them and the remaining 1024-col groups.
            xh = {}     # (g, h) -> 512-col tile for split groups
            xg = {}     # g -> 1024-col tile
            for g in SPLIT_GROUPS:
                for hh in range(2):
                    t_ = xsplit.tile([128, CHUNK], f16, tag=f"x{g}{hh}",
                                     name=f"x{g}{hh}")
                    nc.sync.dma_start(
                        t_[:], xt[:, g * GW + hh * CHUNK:
                                  g * GW + (hh + 1) * CHUNK])
                    xh[(g, hh)] = t_
            wpt = cpool.tile([128, 8], f32)
            nc.sync.dma_start(wpt[:], wp[:, :])
            g3c = wpt[:, 4:5]
            apc = wpt[:, 5:6]
            for g in range(len(SPLIT_GROUPS), NG):
                t_ = xpool.tile([128, GW], f16, tag=f"x{g}", name=f"x{g}")
                nc.sync.dma_start(t_[:], xt[:, g * GW:(g + 1) * GW])
                xg[g] = t_

            # --- diag stack on the ScalarE queue, ahead of the ACT table
            # loads: order 0=W0(silu) 1=B'(xc) 2=G1(M1) 3=G2(M2) 4=G3(M3)
            wpdt = cpool.tile([128, 5, E_SHARD], f16)
            nc.scalar.dma_start(wpdt[:], wpd[:, :])
            diag = wpdt

            # --- warmups: each op on its own scratch (no hazards) so they
            # run during the x0 DMA wait: PE clock gate opens, both ACT
            # table sets load, DVE/GpSimd wake up
            nc.scalar.activation(dum[:, 0:1], dum[:, 0:1], AF.Silu)
            nc.scalar.activation(dum[:, 1:2], dum[:, 1:2], AF.Identity)
            nc.vector.tensor_scalar(dum[:, 2:3], dum[:, 2:3],
                                    1.0, -1.0, OP.min, OP.max)
            nc.gpsimd.tensor_scalar(dum[:, 5:6], dum[:, 5:6],
                                    1.0, -1.0, OP.min, OP.max)
            nc.gpsimd.memset(scr[:], 0.0)
            ps_warm = wpool.tile([128, CHUNK], f32, tag="pswarm", name="pswarm")
            for _ in range(N_WARM):
                nc.tensor.matmul(ps_warm[:], scr[:, 0:128], scr[:],
                                 start=True, stop=True, skip_group_check=True)

            feats = {}      # subgroup -> feature tiles (split: per-half lists)

            def emit_features(g):
                if g in SPLIT_GROUPS:
                    sil, xc, m1, m2, m3 = [], [], [], [], []
                    for hh in range(2):
                        xv = xh[(g, hh)][:]
                        s_ = fsplit.tile([128, CHUNK], f16, tag="fs",
                                         name=f"sil{g}{hh}")
                        nc.scalar.activation(s_[:], xv, AF.Silu)
                        sil.append(s_)
                        c_ = fsplit.tile([128, CHUNK], f16, tag="fs",
                                         name=f"xc{g}{hh}")
                        nc.vector.tensor_scalar(c_[:], xv, 1.0, -1.0,
                                                OP.min, OP.max)
                        xc.append(c_)
                        a_ = fsplit.tile([128, CHUNK], f16, tag="fs",
                                         name=f"m1{g}{hh}")
                        nc.vector.tensor_scalar(a_[:], xv, 1.0, -0.5,
                                                OP.min, OP.max)
                        m1.append(a_)
                        b_ = fsplit.tile([128, CHUNK], f16, tag="fs",
                                         name=f"m2{g}{hh}")
                        nc.vector.tensor_scalar(b_[:], xv, 1.0, 0.0,
                                                OP.min, OP.max)
                        m2.append(b_)
                        d_ = fsplit.tile([128, CHUNK], f16, tag="fs",
                                         name=f"m3{g}{hh}")
                        nc.vector.tensor_scalar(d_[:], xv, 1.0, 0.5,
                                                OP.min, OP.max)
                        m3.append(d_)
                    feats[g] = (sil, xc, m1, m2, m3)
                    return
                xv = xg[g][:]
                sil = spool.tile([128, GW], f16, tag="sil", name=f"sil{g}")
                nc.scalar.activation(sil[:], xv, AF.Silu)
                xc = xcpool.tile([128, GW], f16, tag="xc", name=f"xc{g}")
                nc.vector.tensor_scalar(xc[:], xv, 1.0, -1.0, OP.min, OP.max)
                m1 = m1pool.tile([128, GW], f16, tag="m1", name=f"m1{g}")
                nc.vector.tensor_scalar(m1[:], xv, 1.0, -0.5, OP.min, OP.max)
                m2 = m2pool.tile([128, GW], f16, tag="m2", name=f"m2{g}")
                nc.gpsimd.tensor_scalar(m2[:], xv, 1.0, 0.0, OP.min, OP.max)
                m3 = m3pool.tile([128, GW], f16, tag="m3", name=f"m3{g}")
                nc.vector.tensor_scalar(m3[:], xv, 1.0, 0.5, OP.min, OP.max)
                feats[g] = (sil, xc, m1, m2, m3)

            yo_cur = [None]

            def emit_subgroup(g):
                """matmuls + evac + store for 1024-col subgroup g."""
                sil, xc, m1, m2, m3 = feats.pop(g)
                if g % 2 == 0:
                    yo_cur[0] = ypool.tile([128, 2 * GW], f16, tag="yo",
                                           name=f"yo{g // 2}")
                yo = yo_cur[0]
                yb = (g % 2) * GW
                last = (g == NG - 1)
                for h in range(2):
                    if g in SPLIT_GROUPS:
                        fsil, fxc, fm1 = sil[h][:], xc[h][:], m1[h][:]
                        fm2, fm3 = m2[h][:], m3[h][:]
                    else:
                        cs = slice(h * CHUNK, (h + 1) * CHUNK)
                        fsil, fxc, fm1 = sil[:, cs], xc[:, cs], m1[:, cs]
                        fm2, fm3 = m2[:, cs], m3[:, cs]
                    if last and h == 1:
                        # the very last chunk takes the long-retired warm bank
                        pc = wpool.tile([128, CHUNK], f32, tag="pswarm",
                                        name="ps_last")[:]
                    else:
                        pc = pspool.tile([128, CHUNK], f32, tag="ps",
                                         name=f"ps{g}_{h}")[:]
                    # last subgroup: h0 evacs on DVE, h1 on ACT (concurrent
                    # short tail; identity is the faster final evac)
                    aff = (g in AFF_GROUPS and not last) or (last and h == 0)
                    nc.tensor.matmul(pc, diag[:, 1, :], fxc,
                                     start=True, stop=False, skip_group_check=True)
                    nc.tensor.matmul(pc, diag[:, 2, :], fm1,
                                     start=False, stop=False, skip_group_check=True)
                    nc.tensor.matmul(pc, diag[:, 3, :], fm2,
                                     start=False, stop=False, skip_group_check=True)
                    if not aff:
                        nc.tensor.matmul(pc, diag[:, 4, :], fm3,
                                         start=False, stop=False,
                                         skip_group_check=True)
                    nc.tensor.matmul(pc, diag[:, 0, :], fsil,
                                     start=False, stop=True, skip_group_check=True)
                    ye = slice(yb + h * CHUNK, yb + (h + 1) * CHUNK)
                    if aff:
                        nc.vector.affine_then_add(yo[:, ye], fm3, pc,
                                                  scale=g3c, bias=apc)
                        if last:
                            nc.sync.dma_start(
                                yt[:, g * GW + h * CHUNK:
                                   g * GW + (h + 1) * CHUNK], yo[:, ye])
                    elif last:
                        # final chunk: 2x256 Identity evacs + 2 stores so the
                        # very last wire piece is small
                        for q in range(2):
                            qs = slice(yb + h * CHUNK + q * 256,
                                       yb + h * CHUNK + (q + 1) * 256)
                            nc.scalar.activation(yo[:, qs], pc[:, q * 256:
                                                               (q + 1) * 256],
                                                 AF.Identity, bias=apc, scale=1.0)
                            nc.sync.dma_start(
                                yt[:, g * GW + h * CHUNK + q * 256:
                                   g * GW + h * CHUNK + (q + 1) * 256],
                                yo[:, qs])
                    else:
                        nc.scalar.activation(yo[:, ye], pc, AF.Identity,
                                             bias=apc, scale=1.0)
                if not last:
                    if g % 2 == 1:
                        nc.sync.dma_start(yt[:, (g - 1) * GW:(g + 1) * GW], yo[:])
                    elif g == NG - 2:
                        # penultimate subgroup stores alone (last is split)
                        nc.sync.dma_start(yt[:, g * GW:(g + 1) * GW],
                                          yo[:, 0:GW])

            # software-pipelined emission: features run two subgroups ahead
            # of the matmul/evac stream so a slow feature op never stalls PE
            emit_features(0)
            emit_features(1)
            for g in range(2, NG):
                emit_features(g)
                emit_subgroup(g - 2)
            emit_subgroup(NG - 2)
            emit_subgroup(NG - 1)
    nc.compile()
    return nc


def _host_prep(X, coeffs, W, b):
    c = coeffs.astype(np.float64)
    Wd = W.astype(np.float64)
    bd = b.astype(np.float64)
    sl = 2.0 * (c[:, 1:] - c[:, :-1])           # [O, 4] segment slopes
    d = sl[:, 1:] - sl[:, :-1]                  # [O, 3] slope deltas at knots
    w1 = Wd[:, 1]
    bprime = w1 * sl[:, 0]
    g = w1[:, None] * d                         # [O, 3]
    app = w1 * (c[:, 0] + sl[:, 0] + 0.5 * d[:, 0] - 0.5 * d[:, 2]) + bd

    wp = np.zeros((OUT_F, 8), dtype=np.float32)
    wp[:, 0] = Wd[:, 0]
    wp[:, 1] = bprime
    wp[:, 2] = g[:, 0]
    wp[:, 3] = g[:, 1]
    wp[:, 4] = g[:, 2]
    wp[:, 5] = app
    return wp


def _in_maps(X, coeffs, W, b):
    wp = _host_prep(X, coeffs, W, b)
    in_maps = []
    for cid in range(N_CORES):
        sl = slice(cid * E_SHARD, (cid + 1) * E_SHARD)
        xt = np.ascontiguousarray(X[:, sl].T.astype(np.float16))
        wpc = np.ascontiguousarray(wp[sl])
        wpd = np.zeros((E_SHARD, 5, E_SHARD), dtype=np.float16)
        idx = np.arange(E_SHARD)
        for f in range(5):
            wpd[idx, f, idx] = wpc[:, f].astype(np.float16)
        in_maps.append({"xt": xt, "wp": wpc,
                        "wpd": np.ascontiguousarray(wpd.reshape(E_SHARD, -1))})
    return in_maps


def kernel(X, coeffs, W, b):
    global _nc_cache
    if _nc_cache is None:
        _nc_cache = _build()
    nc = _nc_cache

    in_maps = _in_maps(X, coeffs, W, b)
    res = run_bass_kernel_spmd(nc, in_maps, core_ids=list(range(N_CORES)))
    Y = np.empty((B, OUT_F), dtype=np.float32)
    for c in range(N_CORES):
        sl = slice(c * E_SHARD, (c + 1) * E_SHARD)
        Y[:, sl] = res.results[c]["yt"].T.astype(np.float32)
    return Y


# revision 22
# speedup vs baseline: 1.0609x; 1.0186x over previous
"""KAN layer (B=8192, IN_F=OUT_F=1024, GRID=5) on 8 Trainium2 cores.

Math: Y[b,o] = W0[o]*silu(x) + spline_o(clip(x,-1,1)) * W1[o] + b[o], x = X[b,o]
(idx_in = arange(O) % IN_F is the identity here since O == IN_F).

Factorization used here (clip-form basis, exact):
  Y = W0*silu(x) + B'*xc + G1*M1 + G2*M2 + G3*M3 + A''
  xc  = clip(x, -1, 1)
  Mj  = clip(x, s_j, 1),  s_j in {-0.5, 0.0, 0.5}
  B'  = w1*sl0;  Gj = w1*(sl_j - sl_{j-1});  sl_g = 2*(c_{g+1}-c_g)
  A'' = w1*(c0 + sl0 + 0.5*d1 - 0.5*d3) + b   (d_j = sl_j - sl_{j-1})

Sharding: EDGES across the 8 cores (128 edges per core, full batch 8192 on
the free dim).  X arrives pre-transposed and cast to fp16 on host
([128 edges, 8192 batch]); output returns as fp16 and is cast back on host
(tolerance is 2e-2; fp16 I/O costs ~1e-3).  The 5 diagonal stationaries are
pre-built on the host ([128, 5, 128] fp16) so no DVE work gates the first
matmul.

Per 512-col chunk (one PSUM bank) the per-edge weighted sum runs on
TensorE as diagonal fp16 matmuls accumulating in PSUM.  Chunks of the
AFF_GROUPS subgroups skip the M3 matmul: their evacuation runs on DVE as
affine_then_add (yo = M3*G3 + A'' + psum); other chunks evacuate on
ScalarE (Identity + per-edge A'' bias) after a 5th matmul.  silu on
ScalarE, xc/M1/M3 on DVE (tensor_scalar, 4x fp16 mode), M2 on GpSimd.

Queue plan: x loads (8 x 1024 cols, x0 first) + y stores ride the Sync
HWDGE queue; wpd (diag stack) + wp ride the ScalarE HWDGE queue ahead of
the ACT table loads.  The Identity table load is deferred until after the
first silu so silu0 isn't blocked behind it.  Warmup matmuls on scratch
fill the PE ramp window before x0 lands; the last subgroup splits its
evacuation across DVE+ACT and its stores in two for a short tail.

Note: the exec time is pinned by the shared ~240GB/s per-core DMA wire
(2MB in + 2MB out = ~17us of wire) plus the serialization chain
stream-end -> evac -> store -> final barrier; measured variants that
reordered stores, split first groups, paced loads, or rebalanced evac
engines all landed at or above this configuration's time.
"""
import sys

for _p in ("/root/.axon_site", "/root/.axon_site/_ro/trn_rl_repo", "/root/.axon_site/_ro/pypackages"):
    if _p not in sys.path:
        sys.path.append(_p)

import numpy as np

import concourse.bacc as bacc
import concourse.tile as tile
from concourse import mybir
from concourse.bass_utils import run_bass_kernel_spmd

B, IN_F, OUT_F, GRID = 8192, 1024, 1024, 5
N_CORES = 8
E_SHARD = OUT_F // N_CORES      # 128 edges per core
NG = 8                          # batch groups of 1024
GW = B // NG                    # group width (1024)
CHUNK = 512                     # one PSUM bank of fp32
N_WARM = 6                      # PE clock-gate warmup matmuls
AFF_GROUPS = (0, 2, 4, 6)       # subgroups evacuated via DVE affine_then_add

_nc_cache = None


def _build():
    f16 = mybir.dt.float16
    f32 = mybir.dt.float32
    AF = mybir.ActivationFunctionType
    OP = mybir.AluOpType
    nc = bacc.Bacc("TRN2", target_bir_lowering=False, debug=False)
    xt = nc.dram_tensor("xt", [E_SHARD, B], f16, kind="ExternalInput").ap()
    wp = nc.dram_tensor("wp", [E_SHARD, 8], f32, kind="ExternalInput").ap()
    wpd = nc.dram_tensor("wpd", [E_SHARD, 5 * E_SHARD], f16,
                         kind="ExternalInput").ap()
    yt = nc.dram_tensor("yt", [E_SHARD, B], f16, kind="ExternalOutput").ap()

    with tile.TileContext(nc) as tc:
        with tc.tile_pool(name="const", bufs=1) as cpool, \
             tc.tile_pool(name="xin", bufs=NG) as xpool, \
             tc.tile_pool(name="fsil", bufs=3) as spool, \
             tc.tile_pool(name="fxc", bufs=3) as xcpool, \
             tc.tile_pool(name="fm1", bufs=3) as m1pool, \
             tc.tile_pool(name="fm2", bufs=3) as m2pool, \
             tc.tile_pool(name="fm3", bufs=3) as m3pool, \
             tc.tile_pool(name="yout", bufs=3) as ypool, \
             tc.tile_pool(name="ps", bufs=7, space="PSUM") as pspool, \
             tc.tile_pool(name="pswarm", bufs=1, space="PSUM") as wpool:

            scr = cpool.tile([128, CHUNK], f16)
            dum = cpool.tile([128, 16], f16)

            # --- input prefetch on the SP queue: x0 FIRST (it gates the
            # whole pipeline), then the remaining 1024-col groups.
            xg = []
            x_t = xpool.tile([128, GW], f16, tag="x0", name="x0")
            nc.sync.dma_start(x_t[:], xt[:, 0:GW])
            xg.append(x_t)
            for g in range(1, NG):
                x_t = xpool.tile([128, GW], f16, tag=f"x{g}", name=f"x{g}")
                nc.sync.dma_start(x_t[:], xt[:, g * GW:(g + 1) * GW])
                xg.append(x_t)

            # --- diag stack + per-edge scalars on the ScalarE queue,
            # ahead of the ACT table loads.  wpd gates the first matmul;
            # wp only gates the first affine evac (~1.5us later).
            wpdt = cpool.tile([128, 5, E_SHARD], f16)
            nc.scalar.dma_start(wpdt[:], wpd[:, :])
            wpt = cpool.tile([128, 8], f32)
            nc.scalar.dma_start(wpt[:], wp[:, :])
            g3c = wpt[:, 4:5]
            apc = wpt[:, 5:6]
            diag = wpdt

            # --- warmups on private scratch (no hazards): memset on DVE
            # (fast, runs at body start), silu table+warm on ACT, clip
            # warms on DVE/GpSimd, N_WARM matmuls to open the PE clock
            # gate while the x0 DMA is in flight.  The Identity table
            # load is deferred (emitted after g0's silu below).
            nc.vector.memset(scr[:], 0.0)
            nc.scalar.activation(dum[:, 0:1], dum[:, 0:1], AF.Silu)
            nc.vector.tensor_scalar(dum[:, 2:3], dum[:, 2:3],
                                    1.0, -1.0, OP.min, OP.max)
            nc.gpsimd.tensor_scalar(dum[:, 5:6], dum[:, 5:6],
                                    1.0, -1.0, OP.min, OP.max)
            ps_warm = wpool.tile([128, CHUNK], f32, tag="pswarm", name="pswarm")
            for _ in range(N_WARM):
                nc.tensor.matmul(ps_warm[:], scr[:, 0:128], scr[:],
                                 start=True, stop=True, skip_group_check=True)

            feats = {}      # subgroup -> feature tiles

            def emit_features(g):
                xv = xg[g][:]
                sil = spool.tile([128, GW], f16, tag="sil", name=f"sil{g}")
                nc.scalar.activation(sil[:], xv, AF.Silu)
                xc = xcpool.tile([128, GW], f16, tag="xc", name=f"xc{g}")
                nc.vector.tensor_scalar(xc[:], xv, 1.0, -1.0, OP.min, OP.max)
                m1 = m1pool.tile([128, GW], f16, tag="m1", name=f"m1{g}")
                nc.vector.tensor_scalar(m1[:], xv, 1.0, -0.5, OP.min, OP.max)
                m2 = m2pool.tile([128, GW], f16, tag="m2", name=f"m2{g}")
                if g > 0:
                    nc.gpsimd.tensor_scalar(m2[:], xv, 1.0, 0.0, OP.min, OP.max)
                else:
                    nc.vector.tensor_scalar(m2[:], xv, 1.0, 0.0, OP.min, OP.max)
                m3 = m3pool.tile([128, GW], f16, tag="m3", name=f"m3{g}")
                nc.vector.tensor_scalar(m3[:], xv, 1.0, 0.5, OP.min, OP.max)
                feats[g] = (sil, xc, m1, m2, m3)

            yo_cur = [None]

            def emit_subgroup(g):
                """matmuls + evac + store for 1024-col subgroup g."""
                sil, xc, m1, m2, m3 = feats.pop(g)
                if g % 2 == 0:
                    yo_cur[0] = ypool.tile([128, 2 * GW], f16, tag="yo",
                                           name=f"yo{g // 2}")
                yo = yo_cur[0]
                yb = (g % 2) * GW
                last = (g == NG - 1)
                aff_g = g in AFF_GROUPS
                for h in range(2):
                    cs = slice(h * CHUNK, (h + 1) * CHUNK)
                    if last and h == 1:
                        # the very last chunk takes the long-retired warm bank
                        pc = wpool.tile([128, CHUNK], f32, tag="pswarm",
                                        name="ps_last")[:]
                    else:
                        pc = pspool.tile([128, CHUNK], f32, tag="ps",
                                         name=f"ps{g}_{h}")[:]
                    # last subgroup: h0 evacs on DVE, h1 on ACT (concurrent
                    # short tail; identity is the faster final evac)
                    aff = (aff_g and not last) or (last and h == 0)
                    nc.tensor.matmul(pc, diag[:, 1, :], xc[:, cs],
                                     start=True, stop=False, skip_group_check=True)
                    nc.tensor.matmul(pc, diag[:, 2, :], m1[:, cs],
                                     start=False, stop=False, skip_group_check=True)
                    nc.tensor.matmul(pc, diag[:, 3, :], m2[:, cs],
                                     start=False, stop=False, skip_group_check=True)
                    if not aff:
                        nc.tensor.matmul(pc, diag[:, 4, :], m3[:, cs],
                                         start=False, stop=False,
                                         skip_group_check=True)
                    nc.tensor.matmul(pc, diag[:, 0, :], sil[:, cs],
                                     start=False, stop=True, skip_group_check=True)
                    ye = slice(yb + h * CHUNK, yb + (h + 1) * CHUNK)
                    if aff:
                        nc.vector.affine_then_add(yo[:, ye], m3[:, cs], pc,
                                                  scale=g3c, bias=apc)
                    else:
                        nc.scalar.activation(yo[:, ye], pc, AF.Identity,
                                             bias=apc, scale=1.0)
                    if last:
                        nc.sync.dma_start(yt[:, g * GW + h * CHUNK:
                                             g * GW + (h + 1) * CHUNK],
                                          yo[:, ye])
                if not last:
                    if g % 2 == 1:
                        nc.sync.dma_start(yt[:, (g - 1) * GW:(g + 1) * GW], yo[:])
                    elif g == NG - 2:
                        # penultimate subgroup stores alone (last is split)
                        nc.sync.dma_start(yt[:, g * GW:(g + 1) * GW],
                                          yo[:, 0:GW])

            # software-pipelined emission: features run two subgroups ahead
            # of the matmul/evac stream so a slow feature op never stalls PE
            emit_features(0)
            # deferred Identity table preload: after silu0 on the ACT queue
            # (first Identity evac is ~1.5us later), so silu0 isn't blocked
            nc.scalar.activation(dum[:, 1:2], dum[:, 1:2], AF.Identity)
            emit_features(1)
            for g in range(2, NG):
                emit_features(g)
                emit_subgroup(g - 2)
            emit_subgroup(NG - 2)
            emit_subgroup(NG - 1)
    nc.compile()
    return nc


def _host_prep(X, coeffs, W, b):
    c = coeffs.astype(np.float64)
    Wd = W.astype(np.float64)
    bd = b.astype(np.float64)
    sl = 2.0 * (c[:, 1:] - c[:, :-1])           # [O, 4] segment slopes
    d = sl[:, 1:] - sl[:, :-1]                  # [O, 3] slope deltas at knots
    w1 = Wd[:, 1]
    bprime = w1 * sl[:, 0]
    g = w1[:, None] * d                         # [O, 3]
    app = w1 * (c[:, 0] + sl[:, 0] + 0.5 * d[:, 0] - 0.5 * d[:, 2]) + bd

    wp = np.zeros((OUT_F, 8), dtype=np.float32)
    wp[:, 0] = Wd[:, 0]
    wp[:, 1] = bprime
    wp[:, 2] = g[:, 0]
    wp[:, 3] = g[:, 1]
    wp[:, 4] = g[:, 2]
    wp[:, 5] = app
    return wp


def _in_maps(X, coeffs, W, b):
    wp = _host_prep(X, coeffs, W, b)
    in_maps = []
    for cid in range(N_CORES):
        sl = slice(cid * E_SHARD, (cid + 1) * E_SHARD)
        xt = np.ascontiguousarray(X[:, sl].T.astype(np.float16))
        wpc = np.ascontiguousarray(wp[sl])
        # pre-built diagonal stationaries: [E, 5, E], order
        # 0=W0(silu) 1=B'(xc) 2=G1(M1) 3=G2(M2) 4=G3(M3)
        wpd = np.zeros((E_SHARD, 5, E_SHARD), dtype=np.float16)
        idx = np.arange(E_SHARD)
        for f in range(5):
            wpd[idx, f, idx] = wpc[:, f].astype(np.float16)
        in_maps.append({"xt": xt, "wp": wpc,
                        "wpd": np.ascontiguousarray(wpd.reshape(E_SHARD, -1))})
    return in_maps


def kernel(X, coeffs, W, b):
    global _nc_cache
    if _nc_cache is None:
        _nc_cache = _build()
    nc = _nc_cache

    in_maps = _in_maps(X, coeffs, W, b)
    res = run_bass_kernel_spmd(nc, in_maps, core_ids=list(range(N_CORES)))
    Y = np.empty((B, OUT_F), dtype=np.float32)
    for c in range(N_CORES):
        sl = slice(c * E_SHARD, (c + 1) * E_SHARD)
        Y[:, sl] = res.results[c]["yt"].T.astype(np.float32)
    return Y
